# revision 16
# baseline (speedup 1.0000x reference)
"""Dark channel prior (15x15 sliding-window min, SAME zero padding) on 8 trn2 cores.

Input  [32, 512, 512, 3] f32, output same shape.
Sharding: pure data parallel, 4 images per core.

Computed in bf16 (monotone min => output = bf16 rounding of exact result,
rel err <= 2^-8, well under the 2e-2 gate).

Negated domain: host sends x' = -x (free during the fp32->bf16 cast) and the
device computes sliding-window MAX; host returns -y'. This lets the GPSIMD
Pool engine join in via native pool_max (TensorTensor/min are not legal on
the Pool engine), splitting elementwise work across DVE + Pool.

Host also pre-transposes each image to [wc=1536, h=512] so the device
pipeline needs only ONE transpose pass:
  load [wc, h] tiles -> vertical max tree along free dim (DVE, + Pool taps)
  -> PE transpose (identity matmul) -> PSUM -> ScalarE copy to [h, wc] tiles
  -> horizontal max tree along free dim (DVE, + Pool taps) -> store interior.

Border outputs (rows/cols within 7 of an edge) include the zero padding; in
negated domain all values are <= 0 so the max there is exactly 0. The output
DRAM buffer is donated zero-initialized (bass2jax zero_outs), so the kernel
never writes borders: it stores only interior rows/cols and skips memsets.

Pool offload (vpool/hpool = number of trailing blocks Pool finishes): for
those blocks DVE computes a window-5 max s5, then Pool forms the window-15
result in one pool_max over 3 taps (s5[j], s5[j+5px], s5[j+10px]).
"""

import sys

sys.path.insert(0, "/opt/trn_rl_repo")

import ml_dtypes
import numpy as np

BF16 = ml_dtypes.bfloat16
N_CORES = 8
B, H, W, C = 32, 512, 512, 3
WC = W * C  # 1536
K = 15
R = K // 2  # 7
IMGS_PER_CORE = B // N_CORES  # 4

_BUILD_CACHE = {}


def _gp_pool_max(nc, mybir, out, in_, tap_stride, taps=2):
    """Emit InstPool(max) on the GPSIMD/Pool engine.

    in_: AP of window-start elements; a trailing tap dim [tap_stride, taps]
    is appended so the innermost (reduced) dim covers the window.
    Equivalent to out[..., j] = max_t in_[..., j + t*tap_stride].
    """
    from concourse import ap_utils
    from concourse.ap import AP

    pairs = [list(p) for p in in_.ap] + [[tap_stride, taps]]
    win = AP(in_.tensor, in_.offset, pairs)
    eng = nc.gpsimd
    in_phys = eng.lower_ap(win)
    nd = len(in_phys.ap)
    if nd != 5:
        new_dims = list(range(1, 6 - nd))
        in_phys.ap = mybir.VecI64Pair(
            ap_utils.expand_dims_ap(in_phys.ap, new_dims)
        )
    return eng.add_instruction(
        mybir.InstPool(
            name=f"I-{eng.bass.next_id()}",
            func=mybir.PoolFunctionType.max,
            ins=[in_phys],
            outs=[eng.lower_ap(out)],
        )
    )


def _emit_image_front(nc, mybir, img, x, ident, pools, vpool):
    """load + vertical pass + transpose + PSUM copy; returns the th tile."""
    AluOp = mybir.AluOpType
    bf16 = mybir.dt.bfloat16
    xp = pools["xp"]
    vmp = pools["vmp"]
    vp = pools["vp"]
    ps = pools["ps"]
    thp = pools["thp"]

    # ---- load transposed image [1536 wc, 512 h] as [128, 12, 512] ----
    xview = x.ap().rearrange("(n p) h -> p n h", p=128)  # [128, 48, 512]
    xv = xp.tile([128, 12, H], bf16, tag="xv", name=f"xv{img}")
    # image 0: small first segment so the v-pass starts as soon as possible
    segs = [(0, 3), (3, 12)] if img == 0 else [(0, 6), (6, 12)]
    for lo, hi in segs:
        nc.sync.dma_start(
            xv[:, lo:hi, :], xview[:, img * 12 + lo : img * 12 + hi, :]
        )

    # ---- vertical pass: sliding max over h (free dim) ----
    # DVE runs the 1,2,4,7 tensor_tensor tree on the first nd blocks; Pool
    # runs the identical tree as 2-tap pool_max on the trailing vpool blocks.
    nd = 12 - vpool
    v2 = vmp.tile([128, 12, 511], bf16, tag="vm", name=f"v2_{img}")
    v4 = vmp.tile([128, 12, 509], bf16, tag="vm", name=f"v4_{img}")
    v8 = vmp.tile([128, 12, 505], bf16, tag="vm", name=f"v8_{img}")
    vout = vp.tile([128, 12, H], bf16, tag="vout", name=f"vout{img}")
    if nd > 0:
        vsegs = [(0, 3), (3, nd)] if img == 0 else [(0, nd)]
        for lo, hi in vsegs:
            s = slice(lo, hi)
            nc.vector.tensor_tensor(v2[:, s, :], xv[:, s, 0:511], xv[:, s, 1:512], AluOp.max)
            nc.vector.tensor_tensor(v4[:, s, :], v2[:, s, 0:509], v2[:, s, 2:511], AluOp.max)
            nc.vector.tensor_tensor(v8[:, s, :], v4[:, s, 0:505], v4[:, s, 4:509], AluOp.max)
        for c0 in range(0, nd, 4):
            c1 = min(c0 + 4, nd)
            nc.vector.tensor_tensor(
                vout[:, c0:c1, 7:505], v8[:, c0:c1, 0:498], v8[:, c0:c1, 7:505],
                AluOp.max,
            )
    if vpool > 0:
        s = slice(nd, 12)
        _gp_pool_max(nc, mybir, v2[:, s, :], xv[:, s, 0:511], 1)
        _gp_pool_max(nc, mybir, v4[:, s, :], v2[:, s, 0:509], 2)
        _gp_pool_max(nc, mybir, v8[:, s, :], v4[:, s, 0:505], 4)
        _gp_pool_max(nc, mybir, vout[:, s, 7:505], v8[:, s, 0:498], 7)
    # vout[:, :, 0:7] and [505:512] are left unwritten (stale) -> those columns
    # become output rows that are never stored.

    # ---- transpose [wc, h] -> [h, wc]: 4 h-blocks x 12 wc-chunks ----
    th = thp.tile([128, 4, WC], bf16, tag="th", name=f"th{img}")
    for b in range(4):
        pt = ps.tile([128, WC], bf16, tag="pt", name=f"pt{img}_{b}")
        for c in range(12):
            nc.tensor.transpose(
                pt[:, 128 * c : 128 * (c + 1)],
                vout[:, c, 128 * b : 128 * (b + 1)],
                ident[:],
            )
        nc.scalar.copy(th[:, b, :], pt[:])
    return th


def _emit_image_back(nc, mybir, img, y, th, pools, hpool):
    """horizontal pass + interior store."""
    AluOp = mybir.AluOpType
    bf16 = mybir.dt.bfloat16
    hmp = pools["hmp"]
    op_ = pools["op_"]

    # ---- horizontal pass: sliding max over w (stride 3 in wc) ----
    # m2 per h-block so DVE starts as soon as block 0's copy lands
    m2 = hmp.tile([128, 4, 1533], bf16, tag="hm", name=f"m2_{img}")
    m4 = hmp.tile([128, 4, 1527], bf16, tag="hm", name=f"m4_{img}")
    m8 = hmp.tile([128, 4, 1515], bf16, tag="hm", name=f"m8_{img}")
    ho = op_.tile([128, 4, WC], bf16, tag="ho", name=f"ho{img}")
    for b in range(4):
        nc.vector.tensor_tensor(
            m2[:, b, :], th[:, b, 0:1533], th[:, b, 3:1536], AluOp.max
        )
    nc.vector.tensor_tensor(m4[:], m2[:, :, 0:1527], m2[:, :, 6:1533], AluOp.max)
    nc.vector.tensor_tensor(m8[:], m4[:, :, 0:1515], m4[:, :, 12:1527], AluOp.max)

    # final stage + store (borders stay zero in the donated buffer); the last
    # image computes finals per h-block so its stores start ASAP (tail), the
    # others use one merged final. Stores ride the Activation HW DGE queue,
    # separate from the loads on the SP queue.
    r0 = img * H
    if img == IMGS_PER_CORE - 1:
        for b in range(4):
            nc.vector.tensor_tensor(
                ho[:, b, 21:1515], m8[:, b, 0:1494], m8[:, b, 21:1515], AluOp.max
            )
            plo = R if b == 0 else 0
            phi = 121 if b == 3 else 128
            rows = r0 + 128 * b
            nc.scalar.dma_start(
                y.ap()[rows + plo : rows + phi, 21:1515],
                ho[plo:phi, b, 21:1515],
            )
    else:
        nc.vector.tensor_tensor(
            ho[:, :, 21:1515], m8[:, :, 0:1494], m8[:, :, 21:1515], AluOp.max
        )
        for b in range(4):
            plo = R if b == 0 else 0
            phi = 121 if b == 3 else 128
            rows = r0 + 128 * b
            nc.scalar.dma_start(
                y.ap()[rows + plo : rows + phi, 21:1515],
                ho[plo:phi, b, 21:1515],
            )


DEFAULT_BUFS = dict(xp=2, vmp=2, vp=2, ps=4, thp=2, hmp=2, op=2)


def _build(
    n_imgs=IMGS_PER_CORE,
    bufs=None,
    n_cores=N_CORES,
    vpool=0,
    hpool=0,
):
    bufs = {**DEFAULT_BUFS, **(bufs or {})}
    key = (n_imgs, tuple(sorted(bufs.items())), n_cores, vpool, hpool)
    if key in _BUILD_CACHE:
        return _BUILD_CACHE[key]

    from contextlib import ExitStack

    import concourse.bacc as bacc
    import concourse.tile as tile
    from concourse import mybir
    from concourse.bass_interp import get_hw_module

    bf16 = mybir.dt.bfloat16

    nc = bacc.Bacc(
        "TRN2", target_bir_lowering=False, debug=False, num_devices=n_cores
    )
    x = nc.dram_tensor("x", [n_imgs * WC, H], bf16, kind="ExternalInput")
    y = nc.dram_tensor("y", [n_imgs * H, WC], bf16, kind="ExternalOutput")
    ident_dram = nc.inline_tensor(
        np.eye(128, dtype=np.float32).astype(BF16), name="ident"
    )

    with tile.TileContext(nc) as tc, ExitStack() as ctx:
        cpool = ctx.enter_context(tc.tile_pool(name="const", bufs=1))
        pools = dict(
            xp=ctx.enter_context(tc.tile_pool(name="xp", bufs=bufs["xp"])),
            vmp=ctx.enter_context(tc.tile_pool(name="vmp", bufs=bufs["vmp"])),
            vp=ctx.enter_context(tc.tile_pool(name="vp", bufs=bufs["vp"])),
            thp=ctx.enter_context(tc.tile_pool(name="thp", bufs=bufs["thp"])),
            hmp=ctx.enter_context(tc.tile_pool(name="hmp", bufs=bufs["hmp"])),
            op_=ctx.enter_context(tc.tile_pool(name="op", bufs=bufs["op"])),
            ps=ctx.enter_context(
                tc.tile_pool(name="ps", bufs=bufs["ps"], space="PSUM")
            ),
        )
        ident = cpool.tile([128, 128], bf16)
        nc.sync.dma_start(ident[:], ident_dram.ap())

        # software-pipelined: image i's h-pass is emitted after image i+1's
        # v-pass so the in-order DVE queue never stalls on transpose latency
        ths = {}
        for img in range(n_imgs):
            ths[img] = _emit_image_front(nc, mybir, img, x, ident, pools, vpool)
            if img - 1 in ths:
                _emit_image_back(nc, mybir, img - 1, y, ths.pop(img - 1), pools, hpool)
        for img in sorted(ths):
            _emit_image_back(nc, mybir, img, y, ths.pop(img), pools, hpool)

    nc.finalize()
    nc.m = get_hw_module(nc.m)
    _BUILD_CACHE[key] = nc
    return nc


def run_sharded(full_input, n_imgs=IMGS_PER_CORE, n_cores=N_CORES, **kw):
    """full_input: [n_imgs*n_cores, H, W, C] f32. Returns (full_output, results)."""
    from concourse.bass_utils import run_bass_kernel_spmd

    build_kw = {k: kw.pop(k) for k in ("vpool", "hpool", "bufs") if k in kw}
    nc = _build(n_imgs=n_imgs, n_cores=n_cores, **build_kw)

    # host prep: negate, bf16, per-image transpose to [wc, h]
    xf = np.ascontiguousarray(full_input, dtype=np.float32).reshape(
        n_cores * n_imgs, H, WC
    )
    xt = (-xf.transpose(0, 2, 1)).astype(BF16)  # [imgs, wc, h], negated
    xs = np.ascontiguousarray(xt).reshape(n_cores, n_imgs * WC, H)
    in_maps = [{"x": xs[i]} for i in range(n_cores)]
    res = run_bass_kernel_spmd(nc, in_maps, list(range(n_cores)), **kw)
    out = np.stack([res.results[i]["y"] for i in range(n_cores)])
    out = -out.astype(np.float32)
    # -0.0 from negating the zero borders; normalize to +0.0
    out += 0.0
    return out.reshape(n_cores * n_imgs, H, W, C), res


def kernel(inputs: np.ndarray) -> np.ndarray:
    out, _ = run_sharded(np.asarray(inputs))
    return out.astype(np.float32)


# revision 17
# speedup vs baseline: 1.0180x; 1.0180x over previous
"""Dark channel prior (15x15 sliding-window min, SAME zero padding) on 8 trn2 cores.

Input  [32, 512, 512, 3] f32, output same shape.
Sharding: pure data parallel, 4 images per core.

Computed in bf16 (monotone min => output = bf16 rounding of exact result,
rel err <= 2^-8, well under the 2e-2 gate).

Negated domain: host sends x' = -x (free during the fp32->bf16 cast) and the
device computes sliding-window MAX; host returns -y'. This lets the GPSIMD
Pool engine join in via native pool_max (TensorTensor/min are not legal on
the Pool engine), splitting elementwise work across DVE + Pool.

Host also pre-transposes each image to [wc=1536, h=512] so the device
pipeline needs only ONE transpose pass:
  load [wc, h] tiles -> vertical max tree along free dim (DVE, + Pool taps)
  -> PE transpose (identity matmul) -> PSUM -> ScalarE copy to [h, wc] tiles
  -> horizontal max tree along free dim (DVE, + Pool taps) -> store interior.

Border outputs (rows/cols within 7 of an edge) include the zero padding; in
negated domain all values are <= 0 so the max there is exactly 0. The output
DRAM buffer is donated zero-initialized (bass2jax zero_outs), so the kernel
never writes borders: it stores only interior rows/cols and skips memsets.

Pool offload (vpool/hpool = number of trailing blocks Pool finishes): for
those blocks DVE computes a window-5 max s5, then Pool forms the window-15
result in one pool_max over 3 taps (s5[j], s5[j+5px], s5[j+10px]).
"""

import sys

sys.path.insert(0, "/opt/trn_rl_repo")

import ml_dtypes
import numpy as np

BF16 = ml_dtypes.bfloat16
N_CORES = 8
B, H, W, C = 32, 512, 512, 3
WC = W * C  # 1536
K = 15
R = K // 2  # 7
IMGS_PER_CORE = B // N_CORES  # 4

_BUILD_CACHE = {}


def _gp_pool_max(nc, mybir, out, in_, tap_stride, taps=2):
    """Emit InstPool(max) on the GPSIMD/Pool engine.

    in_: AP of window-start elements; a trailing tap dim [tap_stride, taps]
    is appended so the innermost (reduced) dim covers the window.
    Equivalent to out[..., j] = max_t in_[..., j + t*tap_stride].
    """
    from concourse import ap_utils
    from concourse.ap import AP

    pairs = [list(p) for p in in_.ap] + [[tap_stride, taps]]
    win = AP(in_.tensor, in_.offset, pairs)
    eng = nc.gpsimd
    in_phys = eng.lower_ap(win)
    nd = len(in_phys.ap)
    if nd != 5:
        new_dims = list(range(1, 6 - nd))
        in_phys.ap = mybir.VecI64Pair(
            ap_utils.expand_dims_ap(in_phys.ap, new_dims)
        )
    return eng.add_instruction(
        mybir.InstPool(
            name=f"I-{eng.bass.next_id()}",
            func=mybir.PoolFunctionType.max,
            ins=[in_phys],
            outs=[eng.lower_ap(out)],
        )
    )


def _emit_image_front(nc, mybir, img, x, ident, pools, vpool):
    """load + vertical pass + transpose + PSUM copy; returns the th tile."""
    AluOp = mybir.AluOpType
    bf16 = mybir.dt.bfloat16
    xp = pools["xp"]
    vmp = pools["vmp"]
    vp = pools["vp"]
    ps = pools["ps"]
    thp = pools["thp"]

    # ---- load transposed image [1536 wc, 512 h] as [128, 12, 512] ----
    xview = x.ap().rearrange("(n p) h -> p n h", p=128)  # [128, 48, 512]
    xv = xp.tile([128, 12, H], bf16, tag="xv", name=f"xv{img}")
    # image 0: small first segment so the v-pass starts as soon as possible
    segs = [(0, 3), (3, 12)] if img == 0 else [(0, 12)]
    for lo, hi in segs:
        nc.sync.dma_start(
            xv[:, lo:hi, :], xview[:, img * 12 + lo : img * 12 + hi, :]
        )

    # ---- vertical pass: sliding max over h (free dim) ----
    # DVE runs the 1,2,4,7 tensor_tensor tree on the first nd blocks; Pool
    # runs the identical tree as 2-tap pool_max on the trailing vpool blocks.
    nd = 12 - vpool
    v2 = vmp.tile([128, 12, 511], bf16, tag="vm", name=f"v2_{img}")
    v4 = vmp.tile([128, 12, 509], bf16, tag="vm", name=f"v4_{img}")
    v8 = vmp.tile([128, 12, 505], bf16, tag="vm", name=f"v8_{img}")
    vout = vp.tile([128, 12, H], bf16, tag="vout", name=f"vout{img}")
    if nd > 0:
        vsegs = [(0, 3), (3, nd)] if img == 0 else [(0, nd)]
        for lo, hi in vsegs:
            s = slice(lo, hi)
            nc.vector.tensor_tensor(v2[:, s, :], xv[:, s, 0:511], xv[:, s, 1:512], AluOp.max)
            nc.vector.tensor_tensor(v4[:, s, :], v2[:, s, 0:509], v2[:, s, 2:511], AluOp.max)
            nc.vector.tensor_tensor(v8[:, s, :], v4[:, s, 0:505], v4[:, s, 4:509], AluOp.max)
        fsegs = [(0, 4), (4, 8), (8, nd)] if img == 0 else [(0, nd)]
        for c0, c1 in fsegs:
            nc.vector.tensor_tensor(
                vout[:, c0:c1, 7:505], v8[:, c0:c1, 0:498], v8[:, c0:c1, 7:505],
                AluOp.max,
            )
    if vpool > 0:
        s = slice(nd, 12)
        _gp_pool_max(nc, mybir, v2[:, s, :], xv[:, s, 0:511], 1)
        _gp_pool_max(nc, mybir, v4[:, s, :], v2[:, s, 0:509], 2)
        _gp_pool_max(nc, mybir, v8[:, s, :], v4[:, s, 0:505], 4)
        _gp_pool_max(nc, mybir, vout[:, s, 7:505], v8[:, s, 0:498], 7)
    # vout[:, :, 0:7] and [505:512] are left unwritten (stale) -> those columns
    # become output rows that are never stored.

    # ---- transpose [wc, h] -> [h, wc]: 4 h-blocks x 12 wc-chunks ----
    th = thp.tile([128, 4, WC], bf16, tag="th", name=f"th{img}")
    for b in range(4):
        pt = ps.tile([128, WC], bf16, tag="pt", name=f"pt{img}_{b}")
        for c in range(12):
            nc.tensor.transpose(
                pt[:, 128 * c : 128 * (c + 1)],
                vout[:, c, 128 * b : 128 * (b + 1)],
                ident[:],
            )
        nc.scalar.copy(th[:, b, :], pt[:])
    return th


def _emit_image_back(nc, mybir, img, y, th, pools, hpool):
    """horizontal pass + interior store."""
    AluOp = mybir.AluOpType
    bf16 = mybir.dt.bfloat16
    hmp = pools["hmp"]
    op_ = pools["op_"]

    # ---- horizontal pass: sliding max over w (stride 3 in wc) ----
    # m2 per h-block so DVE starts as soon as block 0's copy lands
    m2 = hmp.tile([128, 4, 1533], bf16, tag="hm", name=f"m2_{img}")
    m4 = hmp.tile([128, 4, 1527], bf16, tag="hm", name=f"m4_{img}")
    m8 = hmp.tile([128, 4, 1515], bf16, tag="hm", name=f"m8_{img}")
    ho = op_.tile([128, 4, WC], bf16, tag="ho", name=f"ho{img}")
    for b in range(4):
        nc.vector.tensor_tensor(
            m2[:, b, :], th[:, b, 0:1533], th[:, b, 3:1536], AluOp.max
        )
    nc.vector.tensor_tensor(m4[:], m2[:, :, 0:1527], m2[:, :, 6:1533], AluOp.max)
    nc.vector.tensor_tensor(m8[:], m4[:, :, 0:1515], m4[:, :, 12:1527], AluOp.max)

    # final stage + store (borders stay zero in the donated buffer); the last
    # image computes finals per h-block so its stores start ASAP (tail), the
    # others use one merged final. Stores ride the Activation HW DGE queue,
    # separate from the loads on the SP queue.
    r0 = img * H
    if img == IMGS_PER_CORE - 1:
        for b in range(4):
            nc.vector.tensor_tensor(
                ho[:, b, 21:1515], m8[:, b, 0:1494], m8[:, b, 21:1515], AluOp.max
            )
            plo = R if b == 0 else 0
            phi = 121 if b == 3 else 128
            rows = r0 + 128 * b
            nc.scalar.dma_start(
                y.ap()[rows + plo : rows + phi, 21:1515],
                ho[plo:phi, b, 21:1515],
            )
    else:
        nc.vector.tensor_tensor(
            ho[:, :, 21:1515], m8[:, :, 0:1494], m8[:, :, 21:1515], AluOp.max
        )
        nc.scalar.dma_start(
            y.ap()[r0 + R : r0 + 128, 21:1515], ho[R:128, 0, 21:1515]
        )
        nc.scalar.dma_start(
            y.ap()[r0 + 128 : r0 + 384, 21:1515].rearrange(
                "(n p) w -> p n w", p=128
            ),
            ho[:, 1:3, 21:1515],
        )
        nc.scalar.dma_start(
            y.ap()[r0 + 384 : r0 + 505, 21:1515], ho[0:121, 3, 21:1515]
        )


DEFAULT_BUFS = dict(xp=2, vmp=2, vp=2, ps=4, thp=2, hmp=2, op=2)


def _build(
    n_imgs=IMGS_PER_CORE,
    bufs=None,
    n_cores=N_CORES,
    vpool=0,
    hpool=0,
):
    bufs = {**DEFAULT_BUFS, **(bufs or {})}
    key = (n_imgs, tuple(sorted(bufs.items())), n_cores, vpool, hpool)
    if key in _BUILD_CACHE:
        return _BUILD_CACHE[key]

    from contextlib import ExitStack

    import concourse.bacc as bacc
    import concourse.tile as tile
    from concourse import mybir
    from concourse.bass_interp import get_hw_module

    bf16 = mybir.dt.bfloat16

    nc = bacc.Bacc(
        "TRN2", target_bir_lowering=False, debug=False, num_devices=n_cores
    )
    x = nc.dram_tensor("x", [n_imgs * WC, H], bf16, kind="ExternalInput")
    y = nc.dram_tensor("y", [n_imgs * H, WC], bf16, kind="ExternalOutput")
    ident_dram = nc.inline_tensor(
        np.eye(128, dtype=np.float32).astype(BF16), name="ident"
    )

    with tile.TileContext(nc) as tc, ExitStack() as ctx:
        cpool = ctx.enter_context(tc.tile_pool(name="const", bufs=1))
        pools = dict(
            xp=ctx.enter_context(tc.tile_pool(name="xp", bufs=bufs["xp"])),
            vmp=ctx.enter_context(tc.tile_pool(name="vmp", bufs=bufs["vmp"])),
            vp=ctx.enter_context(tc.tile_pool(name="vp", bufs=bufs["vp"])),
            thp=ctx.enter_context(tc.tile_pool(name="thp", bufs=bufs["thp"])),
            hmp=ctx.enter_context(tc.tile_pool(name="hmp", bufs=bufs["hmp"])),
            op_=ctx.enter_context(tc.tile_pool(name="op", bufs=bufs["op"])),
            ps=ctx.enter_context(
                tc.tile_pool(name="ps", bufs=bufs["ps"], space="PSUM")
            ),
        )
        ident = cpool.tile([128, 128], bf16)
        nc.sync.dma_start(ident[:], ident_dram.ap())

        # software-pipelined: image i's h-pass is emitted after image i+1's
        # v-pass so the in-order DVE queue never stalls on transpose latency
        ths = {}
        for img in range(n_imgs):
            ths[img] = _emit_image_front(nc, mybir, img, x, ident, pools, vpool)
            if img - 1 in ths:
                _emit_image_back(nc, mybir, img - 1, y, ths.pop(img - 1), pools, hpool)
        for img in sorted(ths):
            _emit_image_back(nc, mybir, img, y, ths.pop(img), pools, hpool)

    nc.finalize()
    nc.m = get_hw_module(nc.m)
    _BUILD_CACHE[key] = nc
    return nc


def run_sharded(full_input, n_imgs=IMGS_PER_CORE, n_cores=N_CORES, **kw):
    """full_input: [n_imgs*n_cores, H, W, C] f32. Returns (full_output, results)."""
    from concourse.bass_utils import run_bass_kernel_spmd

    build_kw = {k: kw.pop(k) for k in ("vpool", "hpool", "bufs") if k in kw}
    nc = _build(n_imgs=n_imgs, n_cores=n_cores, **build_kw)

    # host prep: negate, bf16, per-image transpose to [wc, h]
    xf = np.ascontiguousarray(full_input, dtype=np.float32).reshape(
        n_cores * n_imgs, H, WC
    )
    xt = (-xf.transpose(0, 2, 1)).astype(BF16)  # [imgs, wc, h], negated
    xs = np.ascontiguousarray(xt).reshape(n_cores, n_imgs * WC, H)
    in_maps = [{"x": xs[i]} for i in range(n_cores)]
    res = run_bass_kernel_spmd(nc, in_maps, list(range(n_cores)), **kw)
    out = np.stack([res.results[i]["y"] for i in range(n_cores)])
    out = -out.astype(np.float32)
    # -0.0 from negating the zero borders; normalize to +0.0
    out += 0.0
    return out.reshape(n_cores * n_imgs, H, W, C), res


def kernel(inputs: np.ndarray) -> np.ndarray:
    out, _ = run_sharded(np.asarray(inputs))
    return out.astype(np.float32)
